# revision 8
# baseline (speedup 1.0000x reference)
"""Trainium2 Bass kernel for nn_CompressiveMemory_57750130262084.

The reference computes (B=8, S=4096, DK=DV=1024):
    sigma  = elu(query) + 1                                  [B,S,DK]
    memory = einsum('bkd,bsv->bkv', swap(sigma), value)      [B,DK,DV]
    z_norm = sum_s sigma                                     [B,DK]
    out    = einsum('bsd,bkv->bsv', sigma, memory)
           / einsum('bsd,bk->bs',  sigma, z_norm)[..., None]

Every einsum uses disjoint summed subscripts, so each factorises into
outer products of independent reductions; the algebra collapses to
    out[b,s,v] = sum_s value[b,s,v]        (exactly; query cancels)

So the kernel is a column-sum of `value` over S, broadcast over S.
Sharding: data-parallel over batch, one NeuronCore per batch element.
Per-core work: read 16 MB, reduce 4096 rows -> 1 row, write 16 MB.

v5 schedule (evolved from v1 @120.3us via traced experiments):
  * SDMA engine 15 is ~12% slower per 4KB packet than engines 0-14
    (181 vs 161ns, both directions). HWDGE descriptor dealing (probed
    empirically): block = ceil(partitions/16); partition-blocks go to
    engines 0..15 in order IF partitions %% block == 0, else the whole
    DMA collapses onto engine 0 (10x disaster). A 120-partition DMA
    therefore puts NOTHING on engine 15. Mix: 28 slots x 128 rows
    (uniform deal) + 4 slots x 120 rows (engines 0-14) + 32 rows on
    partitions 0-31 => engine 15 ~40.9us busy vs ~41.5us for the
    rest. Row->partition assignment is free (everything is summed /
    all output rows are identical). APs keep an interleave dim
    between partition and column so the AP optimizer cannot merge a
    contiguous [p][m] into one giant descriptor.
  * All chunk folding on the DVE (tensor_add chains, ~1.23us/chunk).
    Per-window partials are converted to bf16 (ACT) and the PE
    reduces them across partitions into accumulating PSUM with
    single-pass bf16 matmuls (~1.2us/window vs ~4.8us for fp32
    LOW_HIGH, which was the old tail bottleneck). bf16 partials add
    ~1.3e-3 relative error; the harness gate is 2e-2.
  * The 120-partition windows and the leftover rows stream EARLY; the
    last-arriving window is a single chunk (no fold -> convert ->
    matmul -> PSUM copy), so the post-stream tail is ~4us.
  * Note: DMA throttling (util-limit 0.5, ~60%% duty) caps sustained
    rates; the final ~2MB of each stream runs ~220 GB/s regardless of
    structure. The schedule minimizes everything else.
"""

import numpy as np

B, S, D = 8, 4096, 1024
P = 128
H = 512                  # PSUM bank width in f32 (matmul N limit)
NA = 28                  # 128-row interleaved chunks (rows [0, 3584))
B0 = NA * P              # 3584: start of the 120-partition region
PB = 120
NB = 4                   # 4 slots x 120 rows (rows [3584, 4064))
L0 = B0 + PB * NB        # 4064: leftover rows -> partitions 0..31
NL = S - L0              # 32

A_WINDOWS = [(0, 5), (5, 10), (10, 15), (15, 20), (20, 25), (25, 27), (27, 28)]
B_WINDOWS = [(0, 2), (2, 4)]
OUT_REP = 7              # chunks per A output DMA (4 x 7 = 28)

_CACHE: dict = {}


def _build_program():
    import concourse.mybir as mybir
    import concourse.tile as tile
    from concourse import bacc

    f32 = mybir.dt.float32
    bf16 = mybir.dt.bfloat16
    nc = bacc.Bacc("TRN2", target_bir_lowering=False, debug=False,
                   num_devices=B, enable_asserts=False)
    v = nc.declare_dram_parameter("value", [S, D], f32, isOutput=False)
    o = nc.declare_dram_parameter("out", [S, D], f32, isOutput=True)

    v_rows = v[0:B0].rearrange("(c p) m -> c p m", p=P)            # [28][128][1024]
    vb = v[B0:L0].rearrange("(n p) m -> p n m", p=PB)              # [120][4][1024]
    o_rows = o[0:B0].rearrange("(i n p) m -> i p n m", i=NA // OUT_REP, n=OUT_REP, p=P)
    ob = o[B0:L0].rearrange("(n p) m -> p n m", p=PB)

    with tile.TileContext(nc) as tc:
        with (
            tc.tile_pool(name="in", bufs=1) as in_pool,
            tc.tile_pool(name="part", bufs=1) as part_pool,
            tc.tile_pool(name="ones", bufs=1) as ones_pool,
            tc.tile_pool(name="bcast", bufs=1) as bcast_pool,
            tc.tile_pool(name="psum", bufs=1, space="PSUM") as psum_pool,
        ):
            ones_b = ones_pool.tile([P, P], bf16, tag="ones_b")
            nc.vector.memset(ones_b[:], 1.0)

            ps = psum_pool.tile([P, D], f32)

            atiles = [
                in_pool.tile([P, (b - a) * D], f32, tag=f"wa{wi}", name=f"wa{wi}")
                for wi, (a, b) in enumerate(A_WINDOWS)
            ]
            btiles = [
                in_pool.tile([P, (b - a) * D], f32, tag=f"wb{wi}", name=f"wb{wi}")
                for wi, (a, b) in enumerate(B_WINDOWS)
            ]
            lt = in_pool.tile([P, D], f32, tag="left")

            # Input DMAs. First A-window, then the small 120-partition
            # and leftover transfers (so their folds clear early), then
            # the remaining A-windows; the last arrival is one chunk.
            # Leftover goes in two column halves (a full [32][1024]
            # would AP-merge into a single descriptor on one engine).
            def issue_a(wi):
                a, b = A_WINDOWS[wi]
                dst = atiles[wi][:].rearrange("p (n m) -> p n m", n=b - a)
                nc.sync.dma_start(dst, v_rows[a:b].rearrange("n p m -> p n m"))

            issue_a(0)
            nc.sync.dma_start(lt[0:NL, 0:H].unsqueeze(1), v[L0:S, 0:H].unsqueeze(1))
            nc.sync.dma_start(lt[0:NL, H:D].unsqueeze(1), v[L0:S, H:D].unsqueeze(1))
            for wi, (a, b) in enumerate(B_WINDOWS):
                dst = btiles[wi][0:PB].rearrange("p (n m) -> p n m", n=b - a)
                nc.sync.dma_start(dst, vb[:, a:b])
            for wi in range(1, len(A_WINDOWS)):
                issue_a(wi)

            # Folds (DVE) -> bf16 convert (ACT) -> partition-reduce (PE).
            mm = []  # (bf16 moving AP, valid partitions)
            for wi, (a, b) in enumerate(B_WINDOWS):
                t = btiles[wi]
                partial = part_pool.tile([P, D], f32, tag=f"pf{wi % 4}", name=f"pf{wi % 4}")
                nc.vector.tensor_add(partial[0:PB], t[0:PB, 0:D], t[0:PB, D : 2 * D])
                pb = part_pool.tile([P, D], bf16, tag=f"pb{wi % 5}", name=f"pb{wi % 5}")
                nc.scalar.copy(pb[0:PB], partial[0:PB])
                mm.append((pb, PB))
            for wi, (a, b) in enumerate(A_WINDOWS):
                t = atiles[wi]
                n = b - a
                k = wi + len(B_WINDOWS)
                if n == 1:
                    src = t
                else:
                    partial = part_pool.tile([P, D], f32, tag=f"pf{k % 4}", name=f"pf{k % 4}")
                    nc.vector.tensor_add(partial[:], t[:, 0:D], t[:, D : 2 * D])
                    for i in range(2, n):
                        nc.vector.tensor_add(partial[:], partial[:], t[:, i * D : (i + 1) * D])
                    if wi == 0:
                        nc.vector.tensor_add(partial[0:NL], partial[0:NL], lt[0:NL])
                    src = partial
                pb = part_pool.tile([P, D], bf16, tag=f"pb{k % 5}", name=f"pb{k % 5}")
                nc.scalar.copy(pb[:], src[:])
                mm.append((pb, P))

            for k, (m_in, np_) in enumerate(mm):
                for h in range(2):
                    nc.tensor.matmul(
                        ps[:, h * H : (h + 1) * H],
                        ones_b[0:np_],
                        m_in[0:np_, h * H : (h + 1) * H],
                        start=(k == 0),
                        stop=(k == len(mm) - 1),
                    )

            # PSUM -> SBUF in parallel halves (DVE + ACT).
            bc = bcast_pool.tile([P, D], f32)
            nc.vector.tensor_copy(bc[:, 0:H], ps[:, 0:H])
            nc.scalar.copy(bc[:, H:D], ps[:, H:D])

            # Output: broadcast bc to all rows with the same skew.
            for i in range(NA // OUT_REP):
                src = bc[:].unsqueeze(1).to_broadcast((P, OUT_REP, D))
                nc.sync.dma_start(o_rows[i], src)
            nc.sync.dma_start(ob, bc[0:PB].unsqueeze(1).to_broadcast((PB, NB, D)))
            nc.sync.dma_start(o[L0:S, 0:H].unsqueeze(1), bc[0:NL, 0:H].unsqueeze(1))
            nc.sync.dma_start(o[L0:S, H:D].unsqueeze(1), bc[0:NL, H:D].unsqueeze(1))

    nc.compile()
    return nc


def _get_program():
    if "nc" not in _CACHE:
        _CACHE["nc"] = _build_program()
    return _CACHE["nc"]


def kernel(query: np.ndarray, value: np.ndarray) -> np.ndarray:
    from concourse.bass_utils import run_bass_kernel_spmd

    del query  # output is exactly independent of query (see module docstring)
    value = np.ascontiguousarray(value, dtype=np.float32)
    assert value.shape == (B, S, D)

    nc = _get_program()
    in_maps = [{"value": value[b]} for b in range(B)]
    try:
        res = run_bass_kernel_spmd(nc, in_maps, list(range(B)))
    except Exception:
        # The tunneled runtime occasionally surfaces a transient
        # NRT_EXEC_UNIT_UNRECOVERABLE on the first dispatch; retry once.
        import time

        time.sleep(2.0)
        res = run_bass_kernel_spmd(nc, in_maps, list(range(B)))
    return np.stack([res.results[b]["out"] for b in range(B)], axis=0)


# revision 10
# speedup vs baseline: 1.0476x; 1.0476x over previous
"""Trainium2 Bass kernel for nn_CompressiveMemory_57750130262084.

The reference computes (B=8, S=4096, DK=DV=1024):
    sigma  = elu(query) + 1                                  [B,S,DK]
    memory = einsum('bkd,bsv->bkv', swap(sigma), value)      [B,DK,DV]
    z_norm = sum_s sigma                                     [B,DK]
    out    = einsum('bsd,bkv->bsv', sigma, memory)
           / einsum('bsd,bk->bs',  sigma, z_norm)[..., None]

Every einsum uses disjoint summed subscripts, so each factorises into
outer products of independent reductions; the algebra collapses to
    out[b,s,v] = sum_s value[b,s,v]        (exactly; query cancels)

So the kernel is a column-sum of `value` over S, broadcast over S.
Sharding: data-parallel over batch, one NeuronCore per batch element.
Per-core work: read 16 MB, reduce 4096 rows -> 1 row, write 16 MB.

v5 schedule (evolved from v1 @120.3us via traced experiments):
  * SDMA engine 15 is ~12% slower per 4KB packet than engines 0-14
    (181 vs 161ns, both directions). HWDGE descriptor dealing (probed
    empirically): block = ceil(partitions/16); partition-blocks go to
    engines 0..15 in order IF partitions %% block == 0, else the whole
    DMA collapses onto engine 0 (10x disaster). A 120-partition DMA
    therefore puts NOTHING on engine 15. Mix: 28 slots x 128 rows
    (uniform deal) + 4 slots x 120 rows (engines 0-14) + 32 rows on
    partitions 0-31 => engine 15 ~40.9us busy vs ~41.5us for the
    rest. Row->partition assignment is free (everything is summed /
    all output rows are identical). APs keep an interleave dim
    between partition and column so the AP optimizer cannot merge a
    contiguous [p][m] into one giant descriptor.
  * All chunk folding on the DVE (tensor_add chains, ~1.23us/chunk).
    Per-window partials are converted to bf16 (ACT) and the PE
    reduces them across partitions into accumulating PSUM with
    single-pass bf16 matmuls (~1.2us/window vs ~4.8us for fp32
    LOW_HIGH, which was the old tail bottleneck). bf16 partials add
    ~1.3e-3 relative error; the harness gate is 2e-2.
  * The 120-partition windows and the leftover rows stream EARLY; the
    last-arriving window is a single chunk (no fold -> convert ->
    matmul -> PSUM copy), so the post-stream tail is ~4us.
  * Note: DMA throttling (util-limit 0.5, ~60%% duty) caps sustained
    rates; the final ~2MB of each stream runs ~220 GB/s regardless of
    structure. The schedule minimizes everything else.
"""

import numpy as np

B, S, D = 8, 4096, 1024
P = 128
H = 512                  # PSUM bank width in f32 (matmul N limit)
NA = 28                  # 128-row interleaved chunks (rows [0, 3584))
B0 = NA * P              # 3584: start of the 120-partition region
PB = 120
NB = 4                   # 4 slots x 120 rows (rows [3584, 4064))
L0 = B0 + PB * NB        # 4064: leftover rows -> partitions 0..31
NL = S - L0              # 32

A_WINDOWS = [(0, 5), (5, 10), (10, 15), (15, 20), (20, 25), (25, 27), (27, 28)]
B_WINDOWS = [(0, 2), (2, 4)]
OUT_REP = 7              # chunks per A output DMA (4 x 7 = 28)

_CACHE: dict = {}


def _build_program():
    import concourse.mybir as mybir
    import concourse.tile as tile
    from concourse import bacc

    f32 = mybir.dt.float32
    bf16 = mybir.dt.bfloat16
    nc = bacc.Bacc("TRN2", target_bir_lowering=False, debug=False,
                   num_devices=B, enable_asserts=False)
    v = nc.declare_dram_parameter("value", [S, D], f32, isOutput=False)
    o = nc.declare_dram_parameter("out", [S, D], f32, isOutput=True)

    v_rows = v[0:B0].rearrange("(c p) m -> c p m", p=P)            # [28][128][1024]
    vb = v[B0:L0].rearrange("(n p) m -> p n m", p=PB)              # [120][4][1024]
    o_rows = o[0:B0].rearrange("(i n p) m -> i p n m", i=NA // OUT_REP, n=OUT_REP, p=P)
    ob = o[B0:L0].rearrange("(n p) m -> p n m", p=PB)

    with tile.TileContext(nc) as tc:
        with (
            tc.tile_pool(name="in", bufs=1) as in_pool,
            tc.tile_pool(name="part", bufs=1) as part_pool,
            tc.tile_pool(name="ones", bufs=1) as ones_pool,
            tc.tile_pool(name="bcast", bufs=1) as bcast_pool,
            tc.tile_pool(name="psum", bufs=1, space="PSUM") as psum_pool,
        ):
            ones_b = ones_pool.tile([P, P], bf16, tag="ones_b")
            nc.vector.memset(ones_b[:], 1.0)

            ps = psum_pool.tile([P, D], f32)

            atiles = [
                in_pool.tile([P, (b - a) * D], f32, tag=f"wa{wi}", name=f"wa{wi}")
                for wi, (a, b) in enumerate(A_WINDOWS)
            ]
            btiles = [
                in_pool.tile([P, (b - a) * D], f32, tag=f"wb{wi}", name=f"wb{wi}")
                for wi, (a, b) in enumerate(B_WINDOWS)
            ]
            lt = in_pool.tile([P, D], f32, tag="left")

            # Input DMAs. First A-window, then the small 120-partition
            # and leftover transfers (so their folds clear early), then
            # the remaining A-windows; the last arrival is one chunk.
            # Leftover goes in two column halves (a full [32][1024]
            # would AP-merge into a single descriptor on one engine).
            def issue_a(wi):
                a, b = A_WINDOWS[wi]
                dst = atiles[wi][:].rearrange("p (n m) -> p n m", n=b - a)
                nc.sync.dma_start(dst, v_rows[a:b].rearrange("n p m -> p n m"))

            issue_a(0)
            nc.sync.dma_start(lt[0:NL, 0:H].unsqueeze(1), v[L0:S, 0:H].unsqueeze(1))
            nc.sync.dma_start(lt[0:NL, H:D].unsqueeze(1), v[L0:S, H:D].unsqueeze(1))
            # Each B transfer is one slot x one column half: 120 x 2KB
            # descriptors -> block=8, partition-aligned engine blocks
            # (a 240-desc DMA deals 15-desc blocks that straddle
            # partition groups and runs ~1.9x slower per descriptor).
            for wi, (a, b) in enumerate(B_WINDOWS):
                for j in range(a, b):
                    row0 = B0 + PB * j
                    for h in range(2):
                        nc.sync.dma_start(
                            btiles[wi][0:PB, (j - a) * D + h * H : (j - a) * D + (h + 1) * H].unsqueeze(1),
                            v[row0 : row0 + PB, h * H : (h + 1) * H].unsqueeze(1),
                        )
            for wi in range(1, len(A_WINDOWS)):
                issue_a(wi)

            # Folds (DVE) -> bf16 convert (ACT) -> partition-reduce (PE).
            mm = []  # (bf16 moving AP, valid partitions)
            for wi, (a, b) in enumerate(B_WINDOWS):
                t = btiles[wi]
                partial = part_pool.tile([P, D], f32, tag=f"pf{wi % 4}", name=f"pf{wi % 4}")
                nc.vector.tensor_add(partial[0:PB], t[0:PB, 0:D], t[0:PB, D : 2 * D])
                pb = part_pool.tile([P, D], bf16, tag=f"pb{wi % 5}", name=f"pb{wi % 5}")
                nc.scalar.copy(pb[0:PB], partial[0:PB])
                mm.append((pb, PB))
            for wi, (a, b) in enumerate(A_WINDOWS):
                t = atiles[wi]
                n = b - a
                k = wi + len(B_WINDOWS)
                if n == 1:
                    src = t
                else:
                    partial = part_pool.tile([P, D], f32, tag=f"pf{k % 4}", name=f"pf{k % 4}")
                    nc.vector.tensor_add(partial[:], t[:, 0:D], t[:, D : 2 * D])
                    for i in range(2, n):
                        nc.vector.tensor_add(partial[:], partial[:], t[:, i * D : (i + 1) * D])
                    if wi == 0:
                        nc.vector.tensor_add(partial[0:NL], partial[0:NL], lt[0:NL])
                    src = partial
                pb = part_pool.tile([P, D], bf16, tag=f"pb{k % 5}", name=f"pb{k % 5}")
                nc.scalar.copy(pb[:], src[:])
                mm.append((pb, P))

            for k, (m_in, np_) in enumerate(mm):
                for h in range(2):
                    nc.tensor.matmul(
                        ps[:, h * H : (h + 1) * H],
                        ones_b[0:np_],
                        m_in[0:np_, h * H : (h + 1) * H],
                        start=(k == 0),
                        stop=(k == len(mm) - 1),
                    )

            # PSUM -> SBUF in parallel halves (DVE + ACT).
            bc = bcast_pool.tile([P, D], f32)
            nc.vector.tensor_copy(bc[:, 0:H], ps[:, 0:H])
            nc.scalar.copy(bc[:, H:D], ps[:, H:D])

            # Output: broadcast bc to all rows with the same skew.
            for i in range(NA // OUT_REP):
                src = bc[:].unsqueeze(1).to_broadcast((P, OUT_REP, D))
                nc.sync.dma_start(o_rows[i], src)
            for j in range(NB):
                row0 = B0 + PB * j
                for h in range(2):
                    nc.sync.dma_start(
                        o[row0 : row0 + PB, h * H : (h + 1) * H].unsqueeze(1),
                        bc[0:PB, h * H : (h + 1) * H].unsqueeze(1),
                    )
            nc.sync.dma_start(o[L0:S, 0:H].unsqueeze(1), bc[0:NL, 0:H].unsqueeze(1))
            nc.sync.dma_start(o[L0:S, H:D].unsqueeze(1), bc[0:NL, H:D].unsqueeze(1))

    nc.compile()
    return nc


def _get_program():
    if "nc" not in _CACHE:
        _CACHE["nc"] = _build_program()
    return _CACHE["nc"]


def kernel(query: np.ndarray, value: np.ndarray) -> np.ndarray:
    from concourse.bass_utils import run_bass_kernel_spmd

    del query  # output is exactly independent of query (see module docstring)
    value = np.ascontiguousarray(value, dtype=np.float32)
    assert value.shape == (B, S, D)

    nc = _get_program()
    in_maps = [{"value": value[b]} for b in range(B)]
    try:
        res = run_bass_kernel_spmd(nc, in_maps, list(range(B)))
    except Exception:
        # The tunneled runtime occasionally surfaces a transient
        # NRT_EXEC_UNIT_UNRECOVERABLE on the first dispatch; retry once.
        import time

        time.sleep(2.0)
        res = run_bass_kernel_spmd(nc, in_maps, list(range(B)))
    return np.stack([res.results[b]["out"] for b in range(B)], axis=0)


# revision 12
# speedup vs baseline: 1.0528x; 1.0049x over previous
"""Trainium2 Bass kernel for nn_CompressiveMemory_57750130262084.

The reference computes (B=8, S=4096, DK=DV=1024):
    sigma  = elu(query) + 1                                  [B,S,DK]
    memory = einsum('bkd,bsv->bkv', swap(sigma), value)      [B,DK,DV]
    z_norm = sum_s sigma                                     [B,DK]
    out    = einsum('bsd,bkv->bsv', sigma, memory)
           / einsum('bsd,bk->bs',  sigma, z_norm)[..., None]

Every einsum uses disjoint summed subscripts, so each factorises into
outer products of independent reductions; the algebra collapses to
    out[b,s,v] = sum_s value[b,s,v]        (exactly; query cancels)

So the kernel is a column-sum of `value` over S, broadcast over S.
Sharding: data-parallel over batch, one NeuronCore per batch element.
Per-core work: read 16 MB, reduce 4096 rows -> 1 row, write 16 MB.

v7 schedule (evolved from v1 @120.3us via traced experiments):
  * SDMA engine 15 is ~12% slower per packet than engines 0-14 (181
    vs 161ns/4KB, both directions). HWDGE deals a DMA's descriptors
    to engines in contiguous blocks of ceil(N/16); the deal is only
    healthy when the blocks align with from-0 partition groups --
    misaligned blocks (e.g. 240 descs -> 15-desc blocks) run ~1.9x
    slower per descriptor, and a non-dealable count (127 partitions)
    collapses onto ONE engine (10x). Layout: 28 interleaved slots x
    128 rows (uniform deal, engine 15 included) + 4 slots x 120 rows
    as 120-descriptor DMAs (block=8, engine 15 gets NOTHING) + 32
    leftover rows on partitions 0-31. Net: engine 15 ~41us busy vs
    ~41.5us for the others, no straggler tail. Row->partition
    assignment is free (everything is summed / output rows are all
    identical).
  * Only 8 DMAs can be in flight (Tile's HWDGE sem lanes), so big
    window DMAs are interleaved with the small 120-partition ones in
    issue order to keep the SDMA rings fed.
  * All folding on the DVE; each window's LAST tensor_add writes a
    bf16 partial directly (inline cast). The PE reduces partials
    across partitions into accumulating PSUM with single-pass bf16
    matmuls (~1.3us/window vs ~4.8 for fp32 LOW_HIGH). bf16 partials
    add ~1.7e-3 relative error; the harness gate is 2e-2.
  * Output is broadcast-source DMAs split into column halves (2KB
    descriptors run at line rate): PSUM bank 0 is copied by the DVE
    and its half-writes start while bank 1 is still being copied by
    the ACT. The post-stream tail is ~5us.
"""

import numpy as np

B, S, D = 8, 4096, 1024
P = 128
H = 512                  # PSUM bank width in f32 (matmul N limit)
NA = 28                  # 128-row interleaved chunks (rows [0, 3584))
B0 = NA * P              # 3584: start of the 120-partition region
PB = 120
NB = 4                   # 4 slots x 120 rows (rows [3584, 4064))
L0 = B0 + PB * NB        # 4064: leftover rows -> partitions 0..31
NL = S - L0              # 32

A_WINDOWS = [(0, 5), (5, 10), (10, 15), (15, 20), (20, 25), (25, 27), (27, 28)]
B_WINDOWS = [(0, 2), (2, 4)]
OUT_REP = 7              # chunks per A output DMA (4 x 7 = 28)

_CACHE: dict = {}


def _build_program():
    import concourse.mybir as mybir
    import concourse.tile as tile
    from concourse import bacc

    f32 = mybir.dt.float32
    bf16 = mybir.dt.bfloat16
    nc = bacc.Bacc("TRN2", target_bir_lowering=False, debug=False,
                   num_devices=B, enable_asserts=False)
    v = nc.declare_dram_parameter("value", [S, D], f32, isOutput=False)
    o = nc.declare_dram_parameter("out", [S, D], f32, isOutput=True)

    v_rows = v[0:B0].rearrange("(c p) m -> c p m", p=P)            # [28][128][1024]
    o_rows = o[0:B0].rearrange("(i n p) m -> i p n m", i=NA // OUT_REP, n=OUT_REP, p=P)

    with tile.TileContext(nc) as tc:
        with (
            tc.tile_pool(name="in", bufs=1) as in_pool,
            tc.tile_pool(name="part", bufs=1) as part_pool,
            tc.tile_pool(name="ones", bufs=1) as ones_pool,
            tc.tile_pool(name="bcast", bufs=1) as bcast_pool,
            tc.tile_pool(name="psum", bufs=1, space="PSUM") as psum_pool,
        ):
            ones_b = ones_pool.tile([P, P], bf16, tag="ones_b")
            nc.vector.memset(ones_b[:], 1.0)

            ps = psum_pool.tile([P, D], f32)

            atiles = [
                in_pool.tile([P, (b - a) * D], f32, tag=f"wa{wi}", name=f"wa{wi}")
                for wi, (a, b) in enumerate(A_WINDOWS)
            ]
            btiles = [
                in_pool.tile([P, (b - a) * D], f32, tag=f"wb{wi}", name=f"wb{wi}")
                for wi, (a, b) in enumerate(B_WINDOWS)
            ]
            lt = in_pool.tile([P, D], f32, tag="left")

            # --- input DMAs ---------------------------------------------
            def issue_a(wi):
                a, b = A_WINDOWS[wi]
                dst = atiles[wi][:].rearrange("p (n m) -> p n m", n=b - a)
                nc.sync.dma_start(dst, v_rows[a:b].rearrange("n p m -> p n m"))

            def issue_b(wi, j, h):
                # one 120-row slot x one column half: 120 x 2KB descs,
                # block=8, partition-aligned, engine 15 skipped.
                a, _ = B_WINDOWS[wi]
                row0 = B0 + PB * j
                nc.sync.dma_start(
                    btiles[wi][0:PB, (j - a) * D + h * H : (j - a) * D + (h + 1) * H].unsqueeze(1),
                    v[row0 : row0 + PB, h * H : (h + 1) * H].unsqueeze(1),
                )

            issue_a(0)
            issue_a(1)
            issue_a(2)
            nc.sync.dma_start(
                lt[0:NL].rearrange("p (h m) -> p h m", h=2),
                v[L0:S].rearrange("p (h m) -> p h m", h=2),
            )
            for (wi, j, h) in [(0, 0, 0), (0, 0, 1), (0, 1, 0), (0, 1, 1)]:
                issue_b(wi, j, h)
            issue_a(3)
            for (wi, j, h) in [(1, 2, 0), (1, 2, 1)]:
                issue_b(wi, j, h)
            issue_a(4)
            for (wi, j, h) in [(1, 3, 0), (1, 3, 1)]:
                issue_b(wi, j, h)
            issue_a(5)
            issue_a(6)

            # --- folds: DVE chains, last add writes bf16 directly -------
            pbs = {}

            def bf16_partial(key):
                t = part_pool.tile([P, D], bf16, tag=f"pb_{key}", name=f"pb_{key}")
                pbs[key] = t
                return t

            # B windows: single add [0:120] -> bf16
            for wi, (a, b) in enumerate(B_WINDOWS):
                t = btiles[wi]
                pb = bf16_partial(f"b{wi}")
                nc.vector.tensor_add(pb[0:PB], t[0:PB, 0:D], t[0:PB, D : 2 * D])
            # A windows
            for wi, (a, b) in enumerate(A_WINDOWS):
                t = atiles[wi]
                n = b - a
                pb = bf16_partial(f"a{wi}")
                if n == 1:
                    nc.scalar.copy(pb[:], t[:])       # ACT cast, DVE stays free
                    continue
                if n == 2:
                    nc.vector.tensor_add(pb[:], t[:, 0:D], t[:, D : 2 * D])
                    continue
                partial = part_pool.tile([P, D], f32, tag=f"pf{wi % 3}", name=f"pf{wi % 3}")
                nc.vector.tensor_add(partial[:], t[:, 0:D], t[:, D : 2 * D])
                if wi == 0:
                    nc.vector.tensor_add(partial[0:NL], partial[0:NL], lt[0:NL])
                for i in range(2, n - 1):
                    nc.vector.tensor_add(partial[:], partial[:], t[:, i * D : (i + 1) * D])
                nc.vector.tensor_add(pb[:], partial[:], t[:, (n - 1) * D : n * D])

            # --- partition reduction (PE, bf16 single-pass) -------------
            mm_order = ["b0", "a0", "a1", "a2", "b1", "a3", "a4", "a5", "a6"]
            for k, key in enumerate(mm_order):
                np_ = PB if key.startswith("b") else P
                m_in = pbs[key]
                for h in range(2):
                    nc.tensor.matmul(
                        ps[:, h * H : (h + 1) * H],
                        ones_b[0:np_],
                        m_in[0:np_, h * H : (h + 1) * H],
                        start=(k == 0),
                        stop=(k == len(mm_order) - 1),
                    )

            # --- PSUM -> SBUF per bank (DVE bank0, ACT bank1) -----------
            bc0 = bcast_pool.tile([P, H], f32, tag="bc0")
            bc1 = bcast_pool.tile([P, H], f32, tag="bc1")
            nc.vector.tensor_copy(bc0[:], ps[:, 0:H])
            nc.scalar.copy(bc1[:], ps[:, H:D])

            # --- output: broadcast column halves ------------------------
            for h, bch in ((0, bc0), (1, bc1)):
                for i in range(NA // OUT_REP):
                    src = bch[:].unsqueeze(1).to_broadcast((P, OUT_REP, H))
                    nc.sync.dma_start(o_rows[i][:, :, h * H : (h + 1) * H], src)
            for j in range(NB):
                row0 = B0 + PB * j
                for h, bch in ((0, bc0), (1, bc1)):
                    nc.sync.dma_start(
                        o[row0 : row0 + PB, h * H : (h + 1) * H].unsqueeze(1),
                        bch[0:PB].unsqueeze(1),
                    )
            for h, bch in ((0, bc0), (1, bc1)):
                nc.sync.dma_start(o[L0:S, h * H : (h + 1) * H].unsqueeze(1), bch[0:NL].unsqueeze(1))

    nc.compile()
    return nc


def _get_program():
    if "nc" not in _CACHE:
        _CACHE["nc"] = _build_program()
    return _CACHE["nc"]


def kernel(query: np.ndarray, value: np.ndarray) -> np.ndarray:
    from concourse.bass_utils import run_bass_kernel_spmd

    del query  # output is exactly independent of query (see module docstring)
    value = np.ascontiguousarray(value, dtype=np.float32)
    assert value.shape == (B, S, D)

    nc = _get_program()
    in_maps = [{"value": value[b]} for b in range(B)]
    try:
        res = run_bass_kernel_spmd(nc, in_maps, list(range(B)))
    except Exception:
        # The tunneled runtime occasionally surfaces a transient
        # NRT_EXEC_UNIT_UNRECOVERABLE on the first dispatch; retry once.
        import time

        time.sleep(2.0)
        res = run_bass_kernel_spmd(nc, in_maps, list(range(B)))
    return np.stack([res.results[b]["out"] for b in range(B)], axis=0)
